# revision 72
# baseline (speedup 1.0000x reference)
"""KV-cached multi-head attention on 8 Trainium2 NeuronCores.

Sharding: 4-way batch (data parallel) x 2-way heads (tensor parallel).
Core c handles batch b = c//2 and head-half h2 = c%2 (8 of 16 heads).

Device kernel design:
  - Q/K/V and output projections in fp8e4 (e4m3) with error
    compensation: each operand is split hi/lo (lo = residual of hi
    quantization; weights pre-scaled x64/x512 so both parts stay in
    e4m3 normal range), and three DoubleRow passes (hi*hi + hi*lo +
    lo*hi) accumulate in fp32 PSUM.  DoubleRow processes a 256-deep
    contraction at 0.5 cyc/row, so the 3-pass scheme runs 1.33x faster
    than bf16 at ~bf16 accuracy (max rel err ~7e-3 end to end).
  - Attention in bf16: scores computed transposed (S^T = K^T.T @ Q^T
    per 128-key tile) into wide 2-bank PSUM tiles, exp on ACT (no max
    subtraction; |scores| <= ~10), P streamed once for PV (O^T
    accumulation) and once for the softmax denominator (ones-column
    matmul).  All 16 (head, token-half) chunks are emitted as ONE
    global score stream (halves alternating per head, diagonal-tile
    pairs first) with PV/denominator matmuls trailing by `lag` pairs,
    so exp latency is always covered by later scores and the
    score->exp->PV pipeline drains exactly once per kernel.
  - Causal mask via one DVE multiply per diagonal tile against a
    precomputed [zeros | upper-tri | ones] mask; score PSUM banks are
    seeded full-width once so exp over skipped regions is stale-finite.
  - V bias added to the new-token V tiles on DVE (cached V has no
    bias, so it cannot be folded into the output bias); Q/K biases
    ride the ACT PSUM->SBUF copies; normalization (1/d) and the fp8
    hi/lo split of the attention output ride the DVE normalize.
  - Out-projection runs as a tail phase at full PE occupancy;
    results stream to HBM in 2-4 block chunks to shrink the DMA tail.
  - V-projection runs as dim-half phases of m-inner sweeps (8 PSUM
    groups live) ordered to match a kp-sliced DMA arrival schedule, so
    the PE starts ~4us into the kernel and never waits on input DMA.
"""

import sys

sys.path.insert(0, "/opt/trn_rl_repo")

import numpy as np
import ml_dtypes

import concourse.bass as bass  # noqa: F401  (registers AP types)
import concourse.mybir as mybir
import concourse.tile as tile
from concourse import bacc
from concourse.bass_utils import run_bass_kernel_spmd

F32 = mybir.dt.float32
BF16 = mybir.dt.bfloat16
F8 = mybir.dt.float8e4
BF = ml_dtypes.bfloat16
E4 = ml_dtypes.float8_e4m3

D = 2048          # model dim
SQ = 1024         # new tokens per batch
SC = 1024         # cached tokens
SKV = SC + SQ     # total keys
HD = 128          # head dim
HLOC = 8          # heads per core
DH = HLOC * HD    # per-core projected dim (1024)
KP = 8            # 256-deep contraction pairs (2048 = 8 x 256)
NCORES = 8
P = 128

WSCALE_K = 64.0   # host pre-scale on Wk/Wv/Wo so fp8 parts stay normal
WSCALE_Q = 512.0  # Wq additionally carries 1/sqrt(HD)

EXP = mybir.ActivationFunctionType.Exp
COPY = mybir.ActivationFunctionType.Copy
IDENT = mybir.ActivationFunctionType.Identity
DR = mybir.MatmulPerfMode.DoubleRow


def _emit(tc, nc, prm):
    with tc.tile_pool(name="res", bufs=1) as res:
        kt = res.tile([P, HLOC, SKV], BF16, name="kt", tag="kt")
        vv = res.tile([P, 16, DH], BF16, name="vv", tag="vv")
        qt = res.tile([P, HLOC, SQ], BF16, name="qt", tag="qt")
        msk = res.tile([P, 4, 512], BF16, name="msk", tag="msk")
        bvb = res.tile([P, DH], BF16, name="bvb", tag="bvb")
        ones = res.tile([P, 1], BF16, name="ones", tag="ones")
        bia = res.tile([P, 16], F32, name="bia", tag="bia")

        nc.vector.memset(ones[:], 1.0)

        # ---------------- projections (fp8 hi/lo DoubleRow) ----------------
        with (
            tc.tile_pool(name="px", bufs=2) as px,
            tc.tile_pool(name="pw", bufs=3) as pw,
            tc.tile_pool(name="pps", bufs=1, space="PSUM") as pps,
        ):
            def xt(name):
                t = px.tile([P, KP, 2, SQ], F8, name=name, tag=name[-1:] + "x")
                nc.sync.dma_start(t[:], prm[name][:])
                return t

            def wt(name, tag, half):
                t = pw.tile([P, KP, 2, 512], F8, name=f"{name}{half}", tag=tag)
                nc.sync.dma_start(
                    t[:], prm[name][:, :, :, 512 * half : 512 * (half + 1)]
                )
                return t

            # V inputs in sweep-consumption order so PE starts ASAP:
            # interleave kp-pair slices of xvh and wvh0 (extra DMAs only
            # cost idle HWDGE slots, not transfer time)
            xvh = px.tile([P, KP, 2, SQ], F8, name="xvh", tag="hx")
            wvh0 = pw.tile([P, KP, 2, 512], F8, name="wvh0", tag="w0")
            for k2, kn in ((0, 1), (1, 1), (2, 2), (4, 2), (6, 2)):
                nc.sync.dma_start(
                    wvh0[:, k2 : k2 + kn, :, :],
                    prm["wvh"][:, k2 : k2 + kn, :, 0:512],
                )
                nc.sync.dma_start(
                    xvh[:, k2 : k2 + kn, :, :], prm["xvh"][:, k2 : k2 + kn, :, :]
                )
            nc.sync.dma_start(bia[:], prm["bias"][:])
            wvl0 = wt("wvl", "w1", 0)
            xvl = xt("xvl")
            wvh1 = wt("wvh", "w0", 1)
            wvl1 = wt("wvl", "w1", 1)
            nc.sync.dma_start(bvb[:], prm["bvb"][:])
            nc.sync.dma_start(msk[:], prm["msk"][:])
            # K prefetch (fresh buffers - land during V compute)
            xkh, xkl = xt("xkh"), xt("xkl")
            wkhA, wklA = wt("wkh", "w0", 0), wt("wkl", "w1", 0)
            # cache / wo loads (no deps, land during projections)
            nc.sync.dma_start(kt[:, :, 0:SC], prm["ckt"][:])
            nc.sync.dma_start(vv[:, 0:8, :], prm["cv"][:])
            # late tiles (rotation waits earlier releases)
            wkhB, wklB = wt("wkh", "w0", 1), wt("wkl", "w1", 1)
            xqh, xql = xt("xqh"), xt("xql")
            wqhA, wqlA = wt("wqh", "w0", 0), wt("wql", "w1", 0)
            wqhB, wqlB = wt("wqh", "w0", 1), wt("wql", "w1", 1)

            # V: out[tok m, dims] = x_v.T @ w_v ; stationary x, moving w.
            # One dim-half at a time: all 8 token-block PSUM groups live
            # (8 banks), m-inner sweeps ordered by DMA arrival so the PE
            # is never waiting on more than the next tensor.
            for d, wh_, wl_ in ((0, wvh0, wvl0), (1, wvh1, wvl1)):
                ps = [
                    pps.tile([P, 512], F32, name=f"vps{m}_{d}", tag=f"pp{m}")
                    for m in range(8)
                ]
                for sw, x_, w_ in ((0, xvh, wh_), (1, xvh, wl_), (2, xvl, wh_)):
                    # sweep 0 of phase 0 in kp-pair chunks matching the
                    # interleaved DMA arrival order
                    kp_chunks = (
                        [(0,), (1,), (2, 3), (4, 5), (6, 7)]
                        if (sw == 0 and d == 0)
                        else [tuple(range(KP))]
                    )
                    for kc in kp_chunks:
                        for m in range(8):
                            for kp in kc:
                                nc.tensor.matmul(
                                    ps[m][:],
                                    x_[:, kp, :, P * m : P * (m + 1)],
                                    w_[:, kp, :, :],
                                    start=(sw == 0 and kp == 0),
                                    stop=(sw == 2 and kp == KP - 1),
                                    perf_mode=DR,
                                )
                for m in range(8):
                    dsl = slice(512 * d, 512 * (d + 1))
                    nc.scalar.activation(
                        vv[:, 8 + m, dsl], ps[m][:], COPY, scale=1.0 / WSCALE_K,
                    )
                    # new-token V bias (cached V has none, so it can't be
                    # folded into the output bias)
                    nc.vector.tensor_add(
                        vv[:, 8 + m, dsl], vv[:, 8 + m, dsl], bvb[:, dsl]
                    )

            # K then Q: out[dim m, toks] = w.T @ x ; stationary w, moving x
            for whA, whB, wlA, wlB, xh_, xl_, dest, col0, bcol, wsc in (
                (wkhA, wkhB, wklA, wklB, xkh, xkl, kt, SC, 0, WSCALE_K),
                (wqhA, wqhB, wqlA, wqlB, xqh, xql, qt, 0, 8, WSCALE_Q),
            ):
                for m in range(8):
                    wh_ = whA if m < 4 else whB
                    wl_ = wlA if m < 4 else wlB
                    msl = slice(P * (m % 4), P * (m % 4 + 1))
                    ps = [
                        pps.tile(
                            [P, 512], F32,
                            name=f"{dest.name}ps{m}_{c}",
                            tag=f"pp{(2 * m + c) % 8}",
                        )
                        for c in range(2)
                    ]
                    for kp in range(KP):
                        first, last = kp == 0, kp == KP - 1
                        for c in range(2):
                            nc.tensor.matmul(
                                ps[c][:], wh_[:, kp, :, msl],
                                xh_[:, kp, :, 512 * c : 512 * (c + 1)],
                                start=first, stop=False, perf_mode=DR,
                            )
                        for c in range(2):
                            nc.tensor.matmul(
                                ps[c][:], wh_[:, kp, :, msl],
                                xl_[:, kp, :, 512 * c : 512 * (c + 1)],
                                start=False, stop=False, perf_mode=DR,
                            )
                        for c in range(2):
                            nc.tensor.matmul(
                                ps[c][:], wl_[:, kp, :, msl],
                                xh_[:, kp, :, 512 * c : 512 * (c + 1)],
                                start=False, stop=last, perf_mode=DR,
                            )
                    for c in range(2):
                        nc.scalar.activation(
                            dest[:, m, col0 + 512 * c : col0 + 512 * c + 512],
                            ps[c][:], IDENT,
                            bias=bia[:, bcol + m : bcol + m + 1],
                            scale=1.0 / wsc,
                        )

        # ---------------- attention + out-projection ----------------
        with (
            tc.tile_pool(name="at_p", bufs=1) as at_p,
            tc.tile_pool(name="stps", bufs=2, space="PSUM") as stps,
            tc.tile_pool(name="ops", bufs=1, space="PSUM") as ops,
            tc.tile_pool(name="dps", bufs=1, space="PSUM") as dps,
            tc.tile_pool(name="ptp", bufs=8) as ptp,
            tc.tile_pool(name="bcp", bufs=2) as bcp,
        ):
            # at in fp8 hi/lo pair-layout [dim-in-pair, pair, which, tok]
            ath = at_p.tile([P, 4, 2, SQ], F8, name="ath", tag="ath")
            atl = at_p.tile([P, 4, 2, SQ], F8, name="atl", tag="atl")
            outs = at_p.tile([P, 16, SQ], BF16, name="outs", tag="outs")
            woh = at_p.tile([P, 4, 2, D], F8, name="woh", tag="woh")
            wol = at_p.tile([P, 4, 2, D], F8, name="wol", tag="wol")
            nc.sync.dma_start(woh[:], prm["woh"][:])
            nc.sync.dma_start(wol[:], prm["wol"][:])
            n_st = [0]  # st-tile instance counter (first 3 = fresh banks)
            n_pt = [0]  # pt-tile instance counter (first 8 = fresh buffers)

            # One global score/exp/PV stream across all heads and both
            # token halves: PV matmuls trail the score stream by `lag`
            # pairs, so each head's trailing exps are hidden under the next
            # head's scores and the pipeline drains exactly once.
            cfgs = {}
            # all softmax denominators share ONE PSUM bank: 4 rotating
            # [1, 512] rows at partition bases 0/32/64/96 (legal matmul
            # tile_position columns), double-buffering both halves' d
            # accumulators at the cost of a single bank
            dd = dps.tile([P, 512], F32, name="dd", tag="dd")

            def get_cfg(h, c):
                if (h, c) not in cfgs:
                    n_full = 8 + 4 * c
                    n_kv = n_full + 4
                    # key tiles in pairs sharing one 2-bank score tile
                    grps = [
                        list(range(i, min(i + 2, n_kv)))
                        for i in range(0, n_kv, 2)
                    ]
                    dr = 32 * ((2 * h + c) % 3)
                    cfgs[(h, c)] = dict(
                        n_full=n_full, grps=grps, n_gr=len(grps),
                        o_ps=ops.tile(
                            [P, 512], F32, name=f"o{h}_{c}", tag=f"o{c}",
                            bufs=2 if c == 1 else 1,
                        ),
                        d_ps=dd[dr : dr + 1, :],
                        pts={}, npv=[0],
                    )
                return cfgs[(h, c)]

            def scores(h, c, gi_):
                f = get_cfg(h, c)
                gr, n_full = f["grps"][gi_], f["n_full"]
                st = stps.tile([P, 1024], F32, name=f"st{h}_{c}_{gi_}", tag="st")
                pt = ptp.tile([P, 1024], BF16, name=f"pt{h}_{c}_{gi_}", tag="pt")
                f["pts"][gi_] = pt
                # diagonal tiles: compute only the valid query range.  The
                # first st tiles (= all rotating bank-pairs) are written
                # full-width so later exp reads of skipped regions always
                # see finite stale scores.
                fresh = n_st[0] < 2
                n_st[0] += 1
                for t, g in enumerate(gr):
                    col = 512 * t
                    j = g - n_full
                    o0 = 0 if (j < 0 or fresh) else 128 * j
                    nc.tensor.matmul(
                        st[:, col + o0 : col + 512],
                        kt[:, h, P * g : P * (g + 1)],
                        qt[:, h, 512 * c + o0 : 512 * (c + 1)],
                        start=True, stop=True,
                    )
                # exp: one full-width instr unless a diagonal tile lets us
                # skip the masked-out columns (safe once every pt buffer has
                # been written full-width, so skipped columns hold finite
                # stale values for the mask multiply)
                pt_fresh = n_pt[0] < 8
                n_pt[0] += 1
                w = 512 * len(gr)
                if pt_fresh or gr[0] < n_full:
                    nc.scalar.activation(pt[:, 0:w], st[:, 0:w], EXP)
                else:
                    for t, g in enumerate(gr):
                        o0 = 128 * (g - n_full)
                        nc.scalar.activation(
                            pt[:, 512 * t + o0 : 512 * (t + 1)],
                            st[:, 512 * t + o0 : 512 * (t + 1)],
                            EXP,
                        )
                for t, g in enumerate(gr):
                    j = g - n_full
                    if j >= 0:
                        col = 512 * t
                        nc.vector.tensor_mul(
                            pt[:, col : col + 512],
                            pt[:, col : col + 512],
                            msk[:, j, :],
                        )

            def normalize(h, c):
                f = cfgs[(h, c)]
                csl = slice(512 * c, 512 * (c + 1))
                rec = bcp.tile([1, 512], F32, name=f"rec{h}_{c}", tag="rec")
                nc.vector.reciprocal(rec[:], f["d_ps"][:])
                bc = bcp.tile([P, 512], F32, name=f"bc{h}_{c}", tag="bc")
                nc.gpsimd.partition_broadcast(bc[:], rec[:])
                # normalize + hi/lo fp8 split of the attention output
                an = bcp.tile([P, 512], F32, name=f"an{h}_{c}", tag="an")
                nc.vector.tensor_mul(an[:], f["o_ps"][:], bc[:])
                ah_s = ath[:, h // 2, h % 2, csl]
                nc.vector.tensor_copy(ah_s, an[:])
                nc.vector.tensor_sub(atl[:, h // 2, h % 2, csl], an[:], ah_s)

            def pv(h, c, gi_):
                f = cfgs[(h, c)]
                pt = f["pts"][gi_]
                gr = f["grps"][gi_]
                pos = f["npv"][0]
                f["npv"][0] += 1
                for t, g in enumerate(gr):
                    col = 512 * t
                    # masked-out query columns of diagonal tiles contribute
                    # exactly zero - skip them.  Safe: the first PV of each
                    # chunk is the j=0 diagonal tile (full width), so its
                    # start=True zeroes the whole accumulator.
                    o0 = 128 * max(g - f["n_full"], 0)
                    first = pos == 0 and t == 0
                    last = pos == f["n_gr"] - 1 and t == len(gr) - 1
                    nc.tensor.matmul(
                        f["o_ps"][:, o0:512], vv[:, g, P * h : P * (h + 1)],
                        pt[:, col + o0 : col + 512],
                        start=first, stop=last,
                    )
                    nc.tensor.matmul(
                        f["d_ps"][:, o0:512], ones[:],
                        pt[:, col + o0 : col + 512],
                        start=first, stop=last,
                    )
                if f["npv"][0] == f["n_gr"]:
                    normalize(h, c)

            def head_seq(h):
                # per head: alternate halves, diagonal pairs first per
                # half; half 0 exhausts early so its normalize chain hides
                # under the half-1 tail
                out = []
                orders = {}
                for c in (0, 1):
                    n_kv = 12 + 4 * c
                    n_gr = (n_kv + 1) // 2
                    n_diag = 2
                    orders[c] = list(range(n_gr - n_diag, n_gr)) + list(
                        range(n_gr - n_diag)
                    )
                for i in range(len(orders[1])):
                    if i < len(orders[0]):
                        out.append((h, 0, orders[0][i]))
                    out.append((h, 1, orders[1][i]))
                return out

            steps = [s for h in range(HLOC) for s in head_seq(h)]
            lag = 3
            for i, s in enumerate(steps):
                scores(*s)
                if i >= lag:
                    pv(*steps[i - lag])
            for i in range(len(steps) - lag, len(steps)):
                pv(*steps[i])

            def emit_op(c, m):
                # one out-projection group (fp8 hi/lo DoubleRow)
                csl = slice(512 * c, 512 * (c + 1))
                opw = stps.tile([P, 1024], F32, name=f"op{m}_{c}", tag="st")
                op = opw[:, 0:512]
                for sw, w_, a_ in ((0, woh, ath), (1, woh, atl), (2, wol, ath)):
                    for kp in range(4):
                        nc.tensor.matmul(
                            op, w_[:, kp, :, P * m : P * (m + 1)],
                            a_[:, kp, :, csl],
                            start=(sw == 0 and kp == 0),
                            stop=(sw == 2 and kp == 3),
                            perf_mode=DR,
                        )
                nc.scalar.activation(outs[:, m, csl], op, COPY, scale=1.0 / WSCALE_K)
                # stream results out as they complete; finer chunks near the
                # end of the second half to shrink the DMA tail
                step = 1 if (c == 1 and m >= 14) else (2 if (c == 1 and m >= 8) else 4)
                if m % step == step - 1:
                    nc.sync.dma_start(
                        prm["outT"][c, :, m - step + 1 : m + 1, :],
                        outs[:, m - step + 1 : m + 1, 512 * c : 512 * (c + 1)],
                    )

            for c in range(2):
                for m in range(16):
                    emit_op(c, m)


def build():
    nc = bacc.Bacc(None, target_bir_lowering=False)
    prm = {}
    for n, shape, dt in (
        ("xqh", [P, KP, 2, SQ], F8),
        ("xql", [P, KP, 2, SQ], F8),
        ("xkh", [P, KP, 2, SQ], F8),
        ("xkl", [P, KP, 2, SQ], F8),
        ("xvh", [P, KP, 2, SQ], F8),
        ("xvl", [P, KP, 2, SQ], F8),
        ("wqh", [P, KP, 2, DH], F8),
        ("wql", [P, KP, 2, DH], F8),
        ("wkh", [P, KP, 2, DH], F8),
        ("wkl", [P, KP, 2, DH], F8),
        ("wvh", [P, KP, 2, DH], F8),
        ("wvl", [P, KP, 2, DH], F8),
        ("woh", [P, 4, 2, D], F8),
        ("wol", [P, 4, 2, D], F8),
        ("ckt", [P, HLOC, SC], BF16),
        ("cv", [P, HLOC, DH], BF16),
        ("bias", [P, 16], F32),
        ("bvb", [P, DH], BF16),
        ("msk", [P, 4, 512], BF16),
    ):
        prm[n] = nc.declare_dram_parameter(n, shape, dt, isOutput=False)
    prm["outT"] = nc.declare_dram_parameter(
        "outT", [2, P, 16, 512], BF16, isOutput=True
    )
    with tile.TileContext(nc) as tc:
        _emit(tc, nc, prm)
    nc.compile()
    return nc


def _pair(a):
    """[K, M] f32 -> [128, K/256, 2, M]: contraction d = 256*kp + 128*i + p."""
    return np.ascontiguousarray(
        a.reshape(a.shape[0] // 256, 2, P, a.shape[1]).transpose(2, 0, 1, 3)
    )


def _hilo(a):
    hi = a.astype(E4)
    lo = (a - hi.astype(np.float32)).astype(E4)
    return _pair(hi), _pair(lo)


def _hblocks(a):
    """[1024, M] -> [128, 8, M] (row r = 128*h + p)."""
    return np.ascontiguousarray(a.reshape(HLOC, P, a.shape[1]).transpose(1, 0, 2))


def make_in_maps(query, key, value, cached_k, cached_v, Wq, bq, Wk, bk, Wv, bv, Wo, bo):
    """Per-core host prep: slice + transpose + hi/lo fp8 split + casts."""
    s = float(np.sqrt(HD))
    # per-diagonal-position mask: [zeros(o0) | upper-tri | ones]
    msk = np.zeros((P, 4, 512), dtype=np.float32)
    tri = np.triu(np.ones((P, P), dtype=np.float32))
    for j in range(4):
        o0 = P * j
        msk[:, j, o0 : o0 + P] = tri
        msk[:, j, o0 + P :] = 1.0
    msk = msk.astype(BF)

    in_maps = []
    for c in range(NCORES):
        b, h2 = c // 2, c % 2
        hs = slice(DH * h2, DH * (h2 + 1))

        xqh, xql = _hilo(np.ascontiguousarray(query[b].T))
        xkh, xkl = _hilo(np.ascontiguousarray(key[b].T))
        xvh, xvl = _hilo(np.ascontiguousarray(value[b].T))
        wqh, wql = _hilo(np.ascontiguousarray(Wq[hs].T) * (WSCALE_Q / s))
        wkh, wkl = _hilo(np.ascontiguousarray(Wk[hs].T) * WSCALE_K)
        wvh, wvl = _hilo(np.ascontiguousarray(Wv[hs].T) * WSCALE_K)
        woh, wol = _hilo(np.ascontiguousarray(Wo[:, hs].T) * WSCALE_K)

        bias = np.empty((P, 16), dtype=np.float32)
        bias[:, 0:8] = bk[hs].reshape(HLOC, P).T
        bias[:, 8:16] = (bq[hs] / s).reshape(HLOC, P).T

        in_maps.append(
            {
                "xqh": xqh, "xql": xql, "xkh": xkh, "xkl": xkl,
                "xvh": xvh, "xvl": xvl,
                "wqh": wqh, "wql": wql, "wkh": wkh, "wkl": wkl,
                "wvh": wvh, "wvl": wvl,
                "woh": woh, "wol": wol,
                "ckt": _hblocks(np.ascontiguousarray(cached_k[b][:, hs].T)).astype(BF),
                "cv": _hblocks(np.ascontiguousarray(cached_v[b][:, hs])).astype(BF),
                "bias": bias,
                "bvb": np.broadcast_to(bv[hs].astype(BF), (P, DH)).copy(),
                "msk": msk,
            }
        )
    return in_maps


_NC_CACHE = []


def get_nc():
    if not _NC_CACHE:
        _NC_CACHE.append(build())
    return _NC_CACHE[0]


def assemble(results, Wo, bv, bo):
    bias_full = bo
    out = np.empty((4, SQ, D), dtype=np.float32)
    for b in range(4):
        acc = (
            results[2 * b]["outT"].astype(np.float32)
            + results[2 * b + 1]["outT"].astype(np.float32)
        )  # [2, 128, 16, 512]
        yt = acc.transpose(2, 1, 0, 3).reshape(D, SQ)
        out[b] = yt.T + bias_full[None, :]
    return out


def kernel(query, key, value, cached_k, cached_v, Wq, bq, Wk, bk, Wv, bv, Wo, bo):
    query = np.asarray(query, dtype=np.float32)
    key = np.asarray(key, dtype=np.float32)
    value = np.asarray(value, dtype=np.float32)
    cached_k = np.asarray(cached_k, dtype=np.float32)
    cached_v = np.asarray(cached_v, dtype=np.float32)
    Wq, bq = np.asarray(Wq, np.float32), np.asarray(bq, np.float32)
    Wk, bk = np.asarray(Wk, np.float32), np.asarray(bk, np.float32)
    Wv, bv = np.asarray(Wv, np.float32), np.asarray(bv, np.float32)
    Wo, bo = np.asarray(Wo, np.float32), np.asarray(bo, np.float32)

    nc = get_nc()
    in_maps = make_in_maps(
        query, key, value, cached_k, cached_v, Wq, bq, Wk, bk, Wv, bv, Wo, bo
    )
    res = run_bass_kernel_spmd(nc, in_maps, list(range(NCORES)))
    return assemble(res.results, Wo, bv, bo)
